# revision 1
# baseline (speedup 1.0000x reference)
"""Trainium2 Bass kernel for nn_CustomLoss (cross-entropy + epoch correction).

Reference semantics:
    logz   = logsumexp(output, axis=1)                 # [N], C=32
    picked = output[i, target[i]]                      # [N]
    init_loss = mean(logz - picked)
    flag   = any((target == 2) & (argmax(output,1) == 3))
    corr   = epoch**-0.65 * 64 + 0.01
    loss   = init_loss + (corr if flag else 0)
    return init_loss if (loss < 0 or loss/init_loss < 0.2) else loss

Sharding: data-parallel along N across 8 cores; no collectives. Per core the
shard is [128 partitions, 65536 f32 cols] (partition p owns 2048 consecutive
rows x 32 classes); tiles are column slices, tapered at both ends.

Engine balance (every engine under the 1.51 ns/col DMA pace -> DMA-bound):
    sync  per tile one x DMA; t32 in 3 chunk DMAs; final packed output
    ACT   e = exp(x) f32->f16, bf16 cast of x cols [0, ca), td pair build
          (3 prologue-ish chunks), ln(S) in 3 batched chunks with accum
    DVE   one-hot H = (iota == td) bf16 pair-strided 2x, f16 sum-tree rows
          [0, rs), full f16 max-tree, eq3 = (e3 == M), flag = eq3 * H2 with
          accum, final psum copy
    Pool  bf16 cast of x cols [ca, w), f16 sum-tree rows [rs, k)
    PE    picked: psum += xh_chunk^T @ H_chunk (bf16); trace = sum x[i, t_i]

Host does the final scalar arithmetic from per-core partials.
"""

from contextlib import ExitStack

import numpy as np

N, C = 2097152, 32
NCORES = 8
P = 128
NSH = N // NCORES            # rows per core
KTOT = NSH // P              # rows per partition (2048)
WTOT = KTOT * C              # f32 cols per partition (65536)

TILE_W = [512, 1536, 2048, 7168, 7168, 7168, 7168, 7168, 7168, 7168,
          7168, 2560, 1024, 512]
assert sum(TILE_W) == WTOT
NT = len(TILE_W)
WMAX = max(TILE_W)
KMAX = WMAX // C

RS_FRAC = 0.42    # DVE's share of sum-tree rows (rest on Pool)

# t32 chunking (tile indices covered by each chunk) and ln chunking
T_CHUNKS = [(0, 3), (3, 5), (5, 7), (7, 9), (9, 11), (11, NT)]
LN_CHUNKS = [(0, 5), (5, 9), (9, 12), (12, NT)]

_CACHE: dict = {}


def _build_nc(rs_frac=RS_FRAC, tile_w=None, dbg_flag_zero=False, dbg_tile=None):
    import concourse.bass as bass
    import concourse.mybir as mybir

    f32 = mybir.dt.float32
    f16 = mybir.dt.float16
    bf16 = mybir.dt.bfloat16
    i32 = mybir.dt.int32
    i16 = mybir.dt.int16
    AF = mybir.ActivationFunctionType
    ALU = mybir.AluOpType

    tw = list(tile_w or TILE_W)
    nt = len(tw)
    col0 = [sum(tw[:i]) for i in range(nt)]
    ks = [w // C for w in tw]
    krow0 = [sum(ks[:i]) for i in range(nt)]
    rs = [ks[i] if i >= nt - 3
          else (0 if i <= 2
                else max(0, min(ks[i], int(round(ks[i] * rs_frac)))))
          for i in range(nt)]


    tch = T_CHUNKS
    lnch = LN_CHUNKS
    n_tch = len(tch)
    n_lnch = len(lnch)
    # rows covered by each t32 chunk
    tch_rows = [(krow0[a], krow0[b - 1] + ks[b - 1]) for a, b in tch]
    tch_max_i32 = max(2 * (r1 - r0) for r0, r1 in tch_rows)
    ln_rows = [(krow0[a], krow0[b - 1] + ks[b - 1]) for a, b in lnch]
    ln_max = max(r1 - r0 for r0, r1 in ln_rows)
    # chunk index of each tile
    tile_tch = [next(ci for ci, (a, b) in enumerate(tch) if a <= i < b)
                for i in range(nt)]
    tch_dma_at = [max(0, tch[c][0] - 2) for c in range(n_tch)]
    tch_td_at = [max(0, tch[c][0] - 1) for c in range(n_tch)]
    split_x = {i for i in range(nt) if tw[i] >= 2048 and i < nt - 2}
    tail0 = nt - 2                 # tiles [tail0, nt) share one xt buffer
    tile_buf = [i % 3 for i in range(nt)]
    tile_xoff = [0] * nt
    for i in range(tail0, nt):
        tile_buf[i] = tail0 % 3
        tile_xoff[i] = col0[i] - col0[tail0]
    # ... and share one et/Ht/eq3 buffer (freeing the tail from B=2 waits)
    tile_eb = [i % 2 for i in range(nt)]
    tile_hb = [i % 3 for i in range(nt)]
    tile_eoff = [0] * nt           # column offset into et/Ht
    tile_koff = [0] * nt           # row offset into eq3
    for i in range(tail0, nt):
        tile_eb[i] = tail0 % 2
        tile_hb[i] = tail0 % 3
        tile_eoff[i] = col0[i] - col0[tail0]
        tile_koff[i] = krow0[i] - krow0[tail0]
    # cumulative dx-sem targets per tile (per buffer b = i % 3)
    dx_half = [0] * nt   # sem value when first half landed (split tiles)
    dx_full = [0] * nt
    cnt = [0, 0, 0]
    for i in range(nt):
        bxi = tile_buf[i]
        if i in split_x:
            dx_half[i] = (cnt[bxi] + 1) * 16
            cnt[bxi] += 2
        elif i <= tail0:
            cnt[bxi] += 1
        dx_full[i] = cnt[bxi] * 16

    B = 2
    BX = 3
    nc = bass.Bass()
    x = nc.declare_dram_parameter("x", [P, WTOT], f32, isOutput=False)
    t32 = nc.declare_dram_parameter("t32", [P, KTOT * 2], i32, isOutput=False)
    # packed output: [0:128) pk psum, [128:256) flag psum, then ln accums
    OW = 256 + n_lnch + nt
    out = nc.declare_dram_parameter("out", [P, OW], f32, isOutput=True)
    dbg = None
    if dbg_tile is not None:
        dbg = nc.declare_dram_parameter("dbg", [P, 3 * KMAX], f32, isOutput=True)

    with ExitStack() as ctx:
        en = ctx.enter_context
        xt = [en(nc.sbuf_tensor(f"xt{j}", [P, WMAX], f32)) for j in range(BX)]
        et = [en(nc.sbuf_tensor(f"et{j}", [P, WMAX], f16)) for j in range(B)]
        tt = [en(nc.sbuf_tensor(f"tt{j}", [P, tch_max_i32], i32)) for j in range(2)]
        td = en(nc.sbuf_tensor("td", [P, KTOT * 2], bf16))
        Ht = [en(nc.sbuf_tensor(f"Ht{j}", [P, WMAX], bf16)) for j in range(3)]
        s16 = en(nc.sbuf_tensor("s16", [P, KMAX * 16], f16))
        s8 = en(nc.sbuf_tensor("s8", [P, KMAX * 8], f16))
        s4 = en(nc.sbuf_tensor("s4", [P, KMAX * 4], f16))
        s2 = en(nc.sbuf_tensor("s2", [P, KMAX * 2], f16))
        m16 = en(nc.sbuf_tensor("m16", [P, KMAX * 16], f16))
        m8 = en(nc.sbuf_tensor("m8", [P, KMAX * 8], f16))
        m4 = en(nc.sbuf_tensor("m4", [P, KMAX * 4], f16))
        m2 = en(nc.sbuf_tensor("m2", [P, KMAX * 2], f16))
        S = en(nc.sbuf_tensor("S", [P, KTOT], f16))
        lnv = en(nc.sbuf_tensor("lnv", [P, ln_max], f16))
        M = en(nc.sbuf_tensor("M", [P, KMAX], f16))
        eq3 = [en(nc.sbuf_tensor(f"eq3{j}", [P, KMAX + 64], bf16)) for j in range(B)]
        fjunk = en(nc.sbuf_tensor("fjunk", [P, KMAX], f16))
        zbuf = en(nc.sbuf_tensor("zbuf", [P, 128], bf16))
        iota_i = en(nc.sbuf_tensor("iota_i", [P, 32], i16))
        iota_h = en(nc.sbuf_tensor("iota_h", [P, 32], bf16))
        out_sb = en(nc.sbuf_tensor("out_sb", [P, OW], f32))
        dbg_sb = en(nc.sbuf_tensor("dbg_sb", [P, 3 * KMAX], f32)) if dbg_tile is not None else None
        psum = en(nc.psum_tensor([128, 128], f32))
        psum2 = en(nc.psum_tensor([128, 128], f32))

        # --- tick tables ------------------------------------------------
        exp_done = [0] * nt
        tdc_done = [0] * n_tch
        H_done = [0] * nt
        sumS_done = [0] * nt
        flag_done = [0] * nt        # also et free on DVE side
        poolL1_done = [0] * nt      # et free on Pool side
        tree_done = [0] * nt

        # ACT emission order: per tile [tdc?] exp cast [ln?]; td chunk c is
        # emitted right before exp of tile tch[c][0]; ln chunk j emitted
        # after cast of tile lnch[j][1] (or at the end).
        sa_t = 0
        ln_after = {}
        for j, (a, b) in enumerate(lnch[:-1]):
            ln_after[b + 1] = j
        for i in range(nt):
            for c in range(n_tch):
                if tch_td_at[c] == i:
                    sa_t += 1; tdc_done[c] = sa_t
            sa_t += 2 if i in split_x else 1
            exp_done[i] = sa_t
            if i + 1 in ln_after:
                sa_t += 1  # ln chunk
        sa_t += 1          # last ln chunk
        sa_t += 1          # psum copy
        sa_final = sa_t

        sv_t = 1                      # H(0) first
        H_done[0] = 1
        for i in range(nt):
            if i + 1 < nt:
                sv_t += 1; H_done[i + 1] = sv_t
            sv_t += 1; sumS_done[i] = sv_t
            sv_t += 1; flag_done[i] = sv_t
        sv_t += 1
        sv_final = sv_t               # psum2 copy

        sp_t = 1  # iota
        for i in range(nt):
            if rs[i] < ks[i]:
                sp_t += 1; poolL1_done[i] = sp_t
                sp_t += 1; tree_done[i] = sp_t
            else:
                poolL1_done[i] = sp_t
                tree_done[i] = sp_t

        with (
            nc.Block() as block,
            nc.semaphore("dtc") as dtc,
            nc.semaphore("dx0") as dx0,
            nc.semaphore("dx1") as dx1,
            nc.semaphore("dx2") as dx2,
            nc.semaphore("sa") as sa,
            nc.semaphore("sv") as sv,
            nc.semaphore("sp") as sp,
            nc.semaphore("spe") as spe,
            nc.semaphore("ds") as ds,
        ):
            dxs = [dx0, dx1, dx2]

            def tree_l1(eng, ebuf, w, k, tmp16, op, r0, r1, sem=None, eo=0):
                if r1 <= r0:
                    if sem is not None:
                        eng.sem_inc(sem, 1)
                    return
                src = ebuf[:, eo : eo + w].rearrange("p (k c) -> p k c", c=32)
                dst = tmp16[:, 0 : k * 16].rearrange("p (k c) -> p k c", c=16)
                ins = eng.tensor_tensor(
                    dst[:, r0:r1], src[:, r0:r1, 0:16], src[:, r0:r1, 16:32],
                    op=op,
                )
                if sem is not None:
                    ins.then_inc(sem, 1)

            def tree_rest(eng, k, tmps, op, r0, r1, final_out, sem=None):
                if r1 <= r0:
                    if sem is not None:
                        eng.sem_inc(sem, 1)
                    return
                width = 8
                cur = tmps[0][:, 0 : k * 16].rearrange("p (k c) -> p k c", c=16)
                for tmp in tmps[1:]:
                    dst = tmp[:, 0 : k * width].rearrange(
                        "p (k c) -> p k c", c=width
                    )
                    eng.tensor_tensor(
                        dst[:, r0:r1],
                        cur[:, r0:r1, 0:width],
                        cur[:, r0:r1, width : 2 * width],
                        op=op,
                    )
                    cur = dst
                    width //= 2
                ins = eng.tensor_tensor(
                    final_out[:, r0:r1],
                    cur[:, r0:r1, 0:1].rearrange("p k c -> p (k c)"),
                    cur[:, r0:r1, 1:2].rearrange("p k c -> p (k c)"),
                    op=op,
                )
                if sem is not None:
                    ins.then_inc(sem, 1)

            @block.sync
            def _(s: bass.BassEngine):
                for i in range(nt):
                    if i > tail0:
                        continue
                    if i >= BX:
                        j = i - BX
                        s.wait_ge(sa, exp_done[j])
                        s.wait_ge(spe, j + 1)
                    bi = tile_buf[i]
                    if i in split_x:
                        h = tw[i] // 2
                        s.dma_start(
                            out=xt[bi][:, 0:h],
                            in_=x[:, col0[i] : col0[i] + h],
                        ).then_inc(dxs[bi], 16)
                        s.dma_start(
                            out=xt[bi][:, h : tw[i]],
                            in_=x[:, col0[i] + h : col0[i] + tw[i]],
                        ).then_inc(dxs[bi], 16)
                    elif i == tail0:
                        tail_w = WTOT - col0[tail0]
                        s.dma_start(
                            out=xt[bi][:, 0:tail_w],
                            in_=x[:, col0[tail0] : WTOT],
                        ).then_inc(dxs[bi], 16)
                    else:
                        s.dma_start(
                            out=xt[bi][:, 0 : tw[i]],
                            in_=x[:, col0[i] : col0[i] + tw[i]],
                        ).then_inc(dxs[bi], 16)
                    for c in range(n_tch):
                        if tch_dma_at[c] == i:
                            # tt scratch reuse: td build two chunks back done
                            if c >= 2:
                                s.wait_ge(sa, tdc_done[c - 2])
                            r0, r1 = tch_rows[c]
                            s.dma_start(
                                out=tt[c % 2][:, 0 : 2 * (r1 - r0)],
                                in_=t32[:, 2 * r0 : 2 * r1],
                            ).then_inc(dtc, 16)
                s.wait_ge(sa, sa_final)
                s.wait_ge(sv, sv_final)
                s.dma_start(out=out[:, :], in_=out_sb[:]).then_inc(ds, 16)
                if dbg_tile is not None:
                    s.dma_start(out=dbg[:, :], in_=dbg_sb[:]).then_inc(ds, 16)
                    s.wait_ge(ds, 32)
                else:
                    s.wait_ge(ds, 16)

            @block.scalar
            def _(sc: bass.BassEngine):
                def emit_ln(j):
                    a, b2 = lnch[j]
                    r0, r1 = ln_rows[j]
                    sc.wait_ge(sv, sumS_done[b2 - 1])
                    sc.wait_ge(sp, tree_done[b2 - 1])
                    sc.activation(
                        lnv[:, 0 : r1 - r0], S[:, r0:r1], AF.Ln,
                        accum_out=out_sb[:, 256 + j : 257 + j],
                    ).then_inc(sa, 1)

                for i in range(nt):
                    b = i % B
                    for c in range(n_tch):
                        if tch_td_at[c] == i:
                            # td chunk build: i32 low words -> bf16 pairs
                            r0, r1 = tch_rows[c]
                            sc.wait_ge(dtc, 16 * (c + 1))
                            sc.activation(
                                td[:, 2 * r0 : 2 * r1].rearrange(
                                    "p (k two) -> p k two", two=2
                                ),
                                tt[c % 2][:, 0 : 2 * (r1 - r0)]
                                .rearrange("p (k two) -> p k two", two=2)
                                [:, :, 0:1]
                                .broadcast_to([P, r1 - r0, 2]),
                                AF.Copy,
                            ).then_inc(sa, 1)
                    bx = tile_buf[i]
                    xo = tile_xoff[i]
                    eb = tile_eb[i]
                    eo = tile_eoff[i]
                    if i in split_x:
                        h = tw[i] // 2
                        sc.wait_ge(dxs[bx], dx_half[i])
                        if i >= B:
                            sc.wait_ge(sv, flag_done[i - B])
                            sc.wait_ge(sp, poolL1_done[i - B])
                        sc.activation(
                            et[eb][:, eo : eo + h],
                            xt[bx][:, xo : xo + h], AF.Exp,
                        ).then_inc(sa, 1)
                        sc.wait_ge(dxs[bx], dx_full[i])
                        sc.activation(
                            et[eb][:, eo + h : eo + tw[i]],
                            xt[bx][:, xo + h : xo + tw[i]], AF.Exp,
                        ).then_inc(sa, 1)
                    else:
                        sc.wait_ge(dxs[bx], dx_full[i])
                        if i >= B and i <= tail0:
                            sc.wait_ge(sv, flag_done[i - B])
                            sc.wait_ge(sp, poolL1_done[i - B])
                        sc.activation(
                            et[eb][:, eo : eo + tw[i]],
                            xt[bx][:, xo : xo + tw[i]], AF.Exp,
                        ).then_inc(sa, 1)
                    if i + 1 in ln_after:
                        emit_ln(ln_after[i + 1])
                emit_ln(n_lnch - 1)
                sc.wait_ge(spe, nt)
                sc.activation(out_sb[:, 0:128], psum[:], AF.Copy).then_inc(
                    sa, 1
                )

            @block.gpsimd
            def _(g: bass.BassEngine):
                g.memzero(zbuf[:])
                g.iota(iota_i[:], pattern=[[1, 32]], base=0, channel_multiplier=0)
                g.tensor_copy(iota_h[:], iota_i[:]).then_inc(sp, 1)
                for i in range(nt):
                    b = i % B
                    k = ks[i]
                    if rs[i] < k:
                        g.wait_ge(sa, exp_done[i])
                        tree_l1(
                            g, et[b], tw[i], k, s16, ALU.add, rs[i], k, sem=sp
                        )
                        tree_rest(
                            g, k, [s16, s8, s4, s2], ALU.add, rs[i], k,
                            S[:, krow0[i] : krow0[i] + k], sem=sp,
                        )

            @block.tensor
            def _(pe: bass.BassEngine):
                n_mm_done = 0
                total_mm = sum(w // 128 for w in tw)
                for i in range(nt):
                    b = i % B
                    bx = tile_buf[i]
                    xo = tile_xoff[i]
                    k = ks[i]
                    eb = tile_eb[i]
                    eo = tile_eoff[i]
                    ko = tile_koff[i]
                    pe.wait_ge(dxs[bx], dx_full[i])
                    pe.wait_ge(sv, H_done[i])
                    xv = (
                        xt[bx][:, xo : xo + tw[i]]
                        .bitcast(bf16)
                        .rearrange("p (w two) -> p w two", two=2)[:, :, 1]
                    )
                    for g_ in range(tw[i] // 128):
                        mm = pe.matmul(
                            psum[:],
                            lhsT=xv[:, g_ * 128 : (g_ + 1) * 128],
                            rhs=Ht[tile_hb[i]][:, eo + g_ * 128 : eo + (g_ + 1) * 128],
                            start=(n_mm_done == 0),
                            stop=(n_mm_done == total_mm - 1),
                        )
                        n_mm_done += 1
                    mm.then_inc(spe, 1)

            @block.vector
            def _(v: bass.BassEngine):
                def emit_H(j):
                    # one-hot H = (iota == td); pair-strided for DVE 2x
                    kj = ks[j]
                    v.wait_ge(sa, tdc_done[tile_tch[j]])
                    if j >= 3 and j <= tail0:
                        v.wait_ge(spe, j - 2)               # Ht[hb] free
                    eoj = tile_eoff[j]
                    v.tensor_tensor(
                        Ht[tile_hb[j]][:, eoj : eoj + tw[j]].rearrange(
                            "p (k s two) -> p k s two", s=16, two=2
                        ),
                        iota_h[:]
                        .rearrange("p (s two) -> p s two", two=2)
                        .unsqueeze(1)
                        .broadcast_to([P, kj, 16, 2]),
                        td[:, 2 * krow0[j] : 2 * (krow0[j] + kj)]
                        .rearrange("p (k two) -> p k two", two=2)
                        .unsqueeze(2)
                        .broadcast_to([P, kj, 16, 2]),
                        op=ALU.is_equal,
                    ).then_inc(sv, 1)

                v.memzero(eq3[0][:])
                v.memzero(eq3[1][:])
                v.memzero(Ht[0][:, 512:1024])
                v.memzero(Ht[1][:, 1536:2048])
                v.wait_ge(sp, 1)  # iota ready
                emit_H(0)
                for i in range(nt):
                    b = tile_eb[i]
                    eo = tile_eoff[i]
                    ko = tile_koff[i]
                    k = ks[i]
                    if i + 1 < nt:
                        emit_H(i + 1)
                    # sum tree rows [0, rs)
                    if i in split_x:
                        kh = k // 2
                        v.wait_ge(sa, exp_done[i] - 1)
                        tree_l1(v, et[b], tw[i], k, s16, ALU.add, 0,
                                min(rs[i], kh), eo=eo)
                        tree_l1(v, et[b], tw[i], k, m16, ALU.max, 0, kh,
                                eo=eo)
                        v.wait_ge(sa, exp_done[i])
                        tree_l1(v, et[b], tw[i], k, s16, ALU.add,
                                min(rs[i], kh), rs[i], eo=eo)
                        tree_l1(v, et[b], tw[i], k, m16, ALU.max, kh, k,
                                eo=eo)
                    else:
                        v.wait_ge(sa, exp_done[i])
                        tree_l1(v, et[b], tw[i], k, s16, ALU.add, 0, rs[i],
                                eo=eo)
                        tree_l1(v, et[b], tw[i], k, m16, ALU.max, 0, k,
                                eo=eo)
                    tree_rest(
                        v, k, [s16, s8, s4, s2], ALU.add, 0, rs[i],
                        S[:, krow0[i] : krow0[i] + k], sem=sv,
                    )
                    tree_rest(v, k, [m16, m8, m4, m2], ALU.max, 0, k, M)
                    # flag indicator: eq3 = (e3 == rowmax); PE reduces it
                    e3d = et[b][:, eo : eo + tw[i]].rearrange(
                        "p (k c) -> p k c", c=32
                    )
                    v.tensor_tensor(
                        eq3[b][:, ko : ko + k], e3d[:, :, 3], M[:, 0:k],
                        op=ALU.is_equal,
                    )
                    v.scalar_tensor_tensor(
                        fjunk[:, 0:k],
                        eq3[b][:, ko : ko + k],
                        1.0,
                        Ht[tile_hb[i]][:, eo : eo + tw[i]].rearrange(
                            "p (k c) -> p k c", c=32
                        )[:, :, 2],
                        op0=ALU.mult,
                        op1=ALU.mult,
                        accum_out=out_sb[
                            :, 256 + n_lnch + i : 257 + n_lnch + i
                        ],
                    ).then_inc(sv, 1)
                    if dbg_tile == i:
                        v.tensor_copy(dbg_sb[:, 0:KMAX], eq3[b][:, 0:KMAX])
                        v.tensor_copy(dbg_sb[:, KMAX : KMAX + k], M[:, 0:k])
                        v.tensor_copy(dbg_sb[:, 2 * KMAX : 2 * KMAX + k],
                                      e3d[:, :, 3])
                v.memzero(out_sb[:, 128:256])
                v.sem_inc(sv, 1)

    return nc


def _get_nc():
    key = "nc"
    if key not in _CACHE:
        _CACHE[key] = _build_nc()
    return _CACHE[key]


def _finish(out_list, epoch) -> np.float32:
    """Host-side final scalar arithmetic from per-core partials."""
    n_lnch = len(LN_CHUNKS)
    lnsum = 0.0
    flagsum = 0.0
    picksum = 0.0
    for o in out_list:
        o64 = o.astype(np.float64)
        picksum += np.trace(o64[:, 0:128])
        lnsum += o64[:, 256 : 256 + n_lnch].sum()
        flagsum += o64[:, 256 + n_lnch : 256 + n_lnch + NT].sum()
    init_loss = (lnsum - picksum) / N
    corr = float(epoch) ** (-0.65) * 64.0 + 0.01
    loss = init_loss + (corr if flagsum > 0.5 else 0.0)
    bad = (loss < 0) or (loss / init_loss < 0.2)
    return np.float32(init_loss if bad else loss)


def kernel(output: np.ndarray, target: np.ndarray, epoch) -> np.ndarray:
    from concourse.bass_utils import run_bass_kernel_spmd

    nc = _get_nc()

    output = np.ascontiguousarray(output, dtype=np.float32)
    target = np.ascontiguousarray(target, dtype=np.int64)

    in_maps = []
    for cid in range(NCORES):
        xs = output[cid * NSH : (cid + 1) * NSH]
        ts = target[cid * NSH : (cid + 1) * NSH]
        in_maps.append(
            {
                "x": xs.reshape(P, KTOT * C),
                "t32": ts.view(np.int32).reshape(P, KTOT * 2),
            }
        )

    res = run_bass_kernel_spmd(nc, in_maps, list(range(NCORES)))
    return _finish([res.results[i]["out"] for i in range(NCORES)], epoch)



# revision 27
# speedup vs baseline: 1.0565x; 1.0565x over previous
"""Trainium2 Bass kernel for nn_CustomLoss (cross-entropy + epoch correction).

Reference semantics:
    logz   = logsumexp(output, axis=1)                 # [N], C=32
    picked = output[i, target[i]]                      # [N]
    init_loss = mean(logz - picked)
    flag   = any((target == 2) & (argmax(output,1) == 3))
    corr   = epoch**-0.65 * 64 + 0.01
    loss   = init_loss + (corr if flag else 0)
    return init_loss if (loss < 0 or loss/init_loss < 0.2) else loss

Sharding: data-parallel along N across 8 cores; no collectives. Per core the
shard is [128 partitions, 65536 f32 cols] (partition p owns 2048 consecutive
rows x 32 classes); tiles are column slices, tapered at both ends.

Engine balance (every engine under the 1.51 ns/col DMA pace -> DMA-bound):
    sync  per tile one x DMA; t32 in 3 chunk DMAs; final packed output
    ACT   e = exp(x) f32->f16, bf16 cast of x cols [0, ca), td pair build
          (3 prologue-ish chunks), ln(S) in 3 batched chunks with accum
    DVE   one-hot H = (iota == td) bf16 pair-strided 2x, f16 sum-tree rows
          [0, rs), full f16 max-tree, eq3 = (e3 == M), flag = eq3 * H2 with
          accum, final psum copy
    Pool  bf16 cast of x cols [ca, w), f16 sum-tree rows [rs, k)
    PE    picked: psum += xh_chunk^T @ H_chunk (bf16); trace = sum x[i, t_i]

Host does the final scalar arithmetic from per-core partials.
"""

from contextlib import ExitStack

import numpy as np

N, C = 2097152, 32
NCORES = 8
P = 128
NSH = N // NCORES            # rows per core
KTOT = NSH // P              # rows per partition (2048)
WTOT = KTOT * C              # f32 cols per partition (65536)

TILE_W = [512, 1536, 2048, 7168, 7168, 7168, 7168, 7168, 7168, 7168,
          7168, 2560, 1024, 512]
assert sum(TILE_W) == WTOT
NT = len(TILE_W)
WMAX = max(TILE_W)
KMAX = WMAX // C

RS_FRAC = 0.36    # DVE's share of sum-tree rows (rest on Pool)

# t32 chunking (tile indices covered by each chunk) and ln chunking
T_CHUNKS = [(0, 3), (3, 5), (5, 7), (7, 9), (9, 11), (11, NT)]
LN_CHUNKS = [(0, 5), (5, 9), (9, 12), (12, NT)]

_CACHE: dict = {}


def _build_nc(rs_frac=RS_FRAC, tile_w=None, dbg_flag_zero=False, dbg_tile=None):
    import concourse.bass as bass
    import concourse.mybir as mybir

    f32 = mybir.dt.float32
    f16 = mybir.dt.float16
    bf16 = mybir.dt.bfloat16
    i32 = mybir.dt.int32
    i16 = mybir.dt.int16
    AF = mybir.ActivationFunctionType
    ALU = mybir.AluOpType

    tw = list(tile_w or TILE_W)
    nt = len(tw)
    col0 = [sum(tw[:i]) for i in range(nt)]
    ks = [w // C for w in tw]
    krow0 = [sum(ks[:i]) for i in range(nt)]
    rs = [ks[i] if i >= nt - 3
          else (0 if i <= 2
                else max(0, min(ks[i], int(round(ks[i] * rs_frac)))))
          for i in range(nt)]


    tch = T_CHUNKS
    lnch = LN_CHUNKS
    n_tch = len(tch)
    n_lnch = len(lnch)
    # rows covered by each t32 chunk
    tch_rows = [(krow0[a], krow0[b - 1] + ks[b - 1]) for a, b in tch]
    tch_max_i32 = max(2 * (r1 - r0) for r0, r1 in tch_rows)
    ln_rows = [(krow0[a], krow0[b - 1] + ks[b - 1]) for a, b in lnch]
    ln_max = max(r1 - r0 for r0, r1 in ln_rows)
    # chunk index of each tile
    tile_tch = [next(ci for ci, (a, b) in enumerate(tch) if a <= i < b)
                for i in range(nt)]
    tch_dma_at = [max(0, tch[c][0] - 2) for c in range(n_tch)]
    tch_td_at = [max(0, tch[c][0] - 1) for c in range(n_tch)]
    split_x = {i for i in range(nt) if tw[i] >= 2048 and i < nt - 2}
    tail0 = nt - 2                 # tiles [tail0, nt) share one xt buffer
    tile_buf = [i % 3 for i in range(nt)]
    tile_xoff = [0] * nt
    for i in range(tail0, nt):
        tile_buf[i] = tail0 % 3
        tile_xoff[i] = col0[i] - col0[tail0]
    # ... and share one et/Ht/eq3 buffer (freeing the tail from B=2 waits)
    tile_eb = [i % 2 for i in range(nt)]
    tile_hb = [i % 3 for i in range(nt)]
    tile_eoff = [0] * nt           # column offset into et/Ht
    tile_koff = [0] * nt           # row offset into eq3
    for i in range(tail0, nt):
        tile_eb[i] = tail0 % 2
        tile_hb[i] = tail0 % 3
        tile_eoff[i] = col0[i] - col0[tail0]
        tile_koff[i] = krow0[i] - krow0[tail0]
    # cumulative dx-sem targets per tile (per buffer b = i % 3)
    dx_half = [0] * nt   # sem value when first half landed (split tiles)
    dx_full = [0] * nt
    cnt = [0, 0, 0]
    for i in range(nt):
        bxi = tile_buf[i]
        if i in split_x:
            dx_half[i] = (cnt[bxi] + 1) * 16
            cnt[bxi] += 2
        elif i <= tail0:
            cnt[bxi] += 1
        dx_full[i] = cnt[bxi] * 16

    B = 2
    BX = 3
    nc = bass.Bass()
    x = nc.declare_dram_parameter("x", [P, WTOT], f32, isOutput=False)
    t8 = nc.declare_dram_parameter("t8", [P, KTOT], mybir.dt.uint8,
                                   isOutput=False)
    # packed output: [0:128) pk psum, [128:256) flag psum, then ln accums
    OW = 256 + n_lnch + nt
    out = nc.declare_dram_parameter("out", [P, OW], f32, isOutput=True)
    dbg = None
    if dbg_tile is not None:
        dbg = nc.declare_dram_parameter("dbg", [P, 3 * KMAX], f32, isOutput=True)

    with ExitStack() as ctx:
        en = ctx.enter_context
        xt = [en(nc.sbuf_tensor(f"xt{j}", [P, WMAX], f32)) for j in range(BX)]
        et = [en(nc.sbuf_tensor(f"et{j}", [P, WMAX], f16)) for j in range(B)]
        t8s = en(nc.sbuf_tensor("t8s", [P, KTOT], mybir.dt.uint8))
        td = en(nc.sbuf_tensor("td", [P, KTOT * 2], bf16))
        Ht = [en(nc.sbuf_tensor(f"Ht{j}", [P, WMAX], bf16)) for j in range(3)]
        s16 = en(nc.sbuf_tensor("s16", [P, KMAX * 16], f16))
        s8 = en(nc.sbuf_tensor("s8", [P, KMAX * 8], f16))
        s4 = en(nc.sbuf_tensor("s4", [P, KMAX * 4], f16))
        s2 = en(nc.sbuf_tensor("s2", [P, KMAX * 2], f16))
        m16 = en(nc.sbuf_tensor("m16", [P, KMAX * 16], f16))
        m8 = en(nc.sbuf_tensor("m8", [P, KMAX * 8], f16))
        m4 = en(nc.sbuf_tensor("m4", [P, KMAX * 4], f16))
        m2 = en(nc.sbuf_tensor("m2", [P, KMAX * 2], f16))
        S = en(nc.sbuf_tensor("S", [P, KTOT], f16))
        lnv = en(nc.sbuf_tensor("lnv", [P, ln_max], f16))
        M = en(nc.sbuf_tensor("M", [P, KMAX], f16))
        eq3 = [en(nc.sbuf_tensor(f"eq3{j}", [P, KMAX + 64], bf16)) for j in range(B)]
        fjunk = en(nc.sbuf_tensor("fjunk", [P, KMAX], f16))
        zbuf = en(nc.sbuf_tensor("zbuf", [P, 128], bf16))
        iota_i = en(nc.sbuf_tensor("iota_i", [P, 32], i16))
        iota_h = en(nc.sbuf_tensor("iota_h", [P, 32], bf16))
        out_sb = en(nc.sbuf_tensor("out_sb", [P, OW], f32))
        dbg_sb = en(nc.sbuf_tensor("dbg_sb", [P, 3 * KMAX], f32)) if dbg_tile is not None else None
        psum = en(nc.psum_tensor([128, 128], f32))
        psum2 = en(nc.psum_tensor([128, 128], f32))

        # --- tick tables ------------------------------------------------
        exp_done = [0] * nt
        tdc_done = [0] * n_tch
        H_done = [0] * nt
        sumS_done = [0] * nt
        flag_done = [0] * nt        # also et free on DVE side
        poolL1_done = [0] * nt      # et free on Pool side
        tree_done = [0] * nt

        # ACT emission order: per tile [tdc?] exp cast [ln?]; td chunk c is
        # emitted right before exp of tile tch[c][0]; ln chunk j emitted
        # after cast of tile lnch[j][1] (or at the end).
        sa_t = 0
        ln_after = {}
        for j, (a, b) in enumerate(lnch[:-1]):
            ln_after[b + 1] = j
        for i in range(nt):
            for c in range(n_tch):
                if tch_td_at[c] == i:
                    sa_t += 1; tdc_done[c] = sa_t
            sa_t += 2 if i in split_x else 1
            exp_done[i] = sa_t
            if i + 1 in ln_after:
                sa_t += 1  # ln chunk
        sa_t += 1          # last ln chunk
        sa_t += 1          # psum copy
        sa_final = sa_t

        sv_t = 1                      # H(0) first
        H_done[0] = 1
        for i in range(nt):
            if i + 1 < nt:
                sv_t += 1; H_done[i + 1] = sv_t
            sv_t += 1; sumS_done[i] = sv_t
            sv_t += 1; flag_done[i] = sv_t
        sv_t += 1
        sv_final = sv_t               # psum2 copy

        sp_t = 1  # iota
        for i in range(nt):
            if rs[i] < ks[i]:
                sp_t += 1; poolL1_done[i] = sp_t
                sp_t += 1; tree_done[i] = sp_t
            else:
                poolL1_done[i] = sp_t
                tree_done[i] = sp_t

        with (
            nc.Block() as block,
            nc.semaphore("dtc") as dtc,
            nc.semaphore("dx0") as dx0,
            nc.semaphore("dx1") as dx1,
            nc.semaphore("dx2") as dx2,
            nc.semaphore("sa") as sa,
            nc.semaphore("sv") as sv,
            nc.semaphore("sp") as sp,
            nc.semaphore("spe") as spe,
            nc.semaphore("ds") as ds,
        ):
            dxs = [dx0, dx1, dx2]

            def tree_l1(eng, ebuf, w, k, tmp16, op, r0, r1, sem=None, eo=0):
                if r1 <= r0:
                    if sem is not None:
                        eng.sem_inc(sem, 1)
                    return
                src = ebuf[:, eo : eo + w].rearrange("p (k c) -> p k c", c=32)
                dst = tmp16[:, 0 : k * 16].rearrange("p (k c) -> p k c", c=16)
                ins = eng.tensor_tensor(
                    dst[:, r0:r1], src[:, r0:r1, 0:16], src[:, r0:r1, 16:32],
                    op=op,
                )
                if sem is not None:
                    ins.then_inc(sem, 1)

            def tree_rest(eng, k, tmps, op, r0, r1, final_out, sem=None):
                if r1 <= r0:
                    if sem is not None:
                        eng.sem_inc(sem, 1)
                    return
                width = 8
                cur = tmps[0][:, 0 : k * 16].rearrange("p (k c) -> p k c", c=16)
                for tmp in tmps[1:]:
                    dst = tmp[:, 0 : k * width].rearrange(
                        "p (k c) -> p k c", c=width
                    )
                    eng.tensor_tensor(
                        dst[:, r0:r1],
                        cur[:, r0:r1, 0:width],
                        cur[:, r0:r1, width : 2 * width],
                        op=op,
                    )
                    cur = dst
                    width //= 2
                ins = eng.tensor_tensor(
                    final_out[:, r0:r1],
                    cur[:, r0:r1, 0:1].rearrange("p k c -> p (k c)"),
                    cur[:, r0:r1, 1:2].rearrange("p k c -> p (k c)"),
                    op=op,
                )
                if sem is not None:
                    ins.then_inc(sem, 1)

            @block.sync
            def _(s: bass.BassEngine):
                for i in range(nt):
                    if i > tail0:
                        continue
                    if i >= BX:
                        j = i - BX
                        s.wait_ge(sa, exp_done[j])
                        s.wait_ge(spe, j + 1)
                    bi = tile_buf[i]
                    if i in split_x:
                        h = tw[i] // 2
                        s.dma_start(
                            out=xt[bi][:, 0:h],
                            in_=x[:, col0[i] : col0[i] + h],
                        ).then_inc(dxs[bi], 16)
                        s.dma_start(
                            out=xt[bi][:, h : tw[i]],
                            in_=x[:, col0[i] + h : col0[i] + tw[i]],
                        ).then_inc(dxs[bi], 16)
                    elif i == tail0:
                        tail_w = WTOT - col0[tail0]
                        s.dma_start(
                            out=xt[bi][:, 0:tail_w],
                            in_=x[:, col0[tail0] : WTOT],
                        ).then_inc(dxs[bi], 16)
                    else:
                        s.dma_start(
                            out=xt[bi][:, 0 : tw[i]],
                            in_=x[:, col0[i] : col0[i] + tw[i]],
                        ).then_inc(dxs[bi], 16)
                    if i == 0:
                        s.dma_start(out=t8s[:, :], in_=t8[:, :]).then_inc(
                            dtc, 16
                        )
                s.wait_ge(sa, sa_final)
                s.wait_ge(sv, sv_final)
                s.dma_start(out=out[:, :], in_=out_sb[:]).then_inc(ds, 16)
                if dbg_tile is not None:
                    s.dma_start(out=dbg[:, :], in_=dbg_sb[:]).then_inc(ds, 16)
                    s.wait_ge(ds, 32)
                else:
                    s.wait_ge(ds, 16)

            @block.scalar
            def _(sc: bass.BassEngine):
                def emit_ln(j):
                    a, b2 = lnch[j]
                    r0, r1 = ln_rows[j]
                    sc.wait_ge(sv, sumS_done[b2 - 1])
                    sc.wait_ge(sp, tree_done[b2 - 1])
                    sc.activation(
                        lnv[:, 0 : r1 - r0], S[:, r0:r1], AF.Ln,
                        accum_out=out_sb[:, 256 + j : 257 + j],
                    ).then_inc(sa, 1)

                for i in range(nt):
                    b = i % B
                    for c in range(n_tch):
                        if tch_td_at[c] == i:
                            # td chunk build: i32 low words -> bf16 pairs
                            r0, r1 = tch_rows[c]
                            sc.wait_ge(dtc, 16)
                            sc.activation(
                                td[:, 2 * r0 : 2 * r1].rearrange(
                                    "p (k two) -> p k two", two=2
                                ),
                                t8s[:, r0:r1].unsqueeze(2).broadcast_to(
                                    [P, r1 - r0, 2]
                                ),
                                AF.Copy,
                            ).then_inc(sa, 1)
                    bx = tile_buf[i]
                    xo = tile_xoff[i]
                    eb = tile_eb[i]
                    eo = tile_eoff[i]
                    if i in split_x:
                        h = tw[i] // 2
                        sc.wait_ge(dxs[bx], dx_half[i])
                        if i >= B:
                            sc.wait_ge(sv, flag_done[i - B])
                            sc.wait_ge(sp, poolL1_done[i - B])
                        sc.activation(
                            et[eb][:, eo : eo + h],
                            xt[bx][:, xo : xo + h], AF.Exp,
                        ).then_inc(sa, 1)
                        sc.wait_ge(dxs[bx], dx_full[i])
                        sc.activation(
                            et[eb][:, eo + h : eo + tw[i]],
                            xt[bx][:, xo + h : xo + tw[i]], AF.Exp,
                        ).then_inc(sa, 1)
                    else:
                        sc.wait_ge(dxs[bx], dx_full[i])
                        if i >= B and i <= tail0:
                            sc.wait_ge(sv, flag_done[i - B])
                            sc.wait_ge(sp, poolL1_done[i - B])
                        sc.activation(
                            et[eb][:, eo : eo + tw[i]],
                            xt[bx][:, xo : xo + tw[i]], AF.Exp,
                        ).then_inc(sa, 1)
                    if i + 1 in ln_after:
                        emit_ln(ln_after[i + 1])
                emit_ln(n_lnch - 1)
                sc.wait_ge(spe, nt)
                sc.activation(out_sb[:, 0:128], psum[:], AF.Copy).then_inc(
                    sa, 1
                )

            @block.gpsimd
            def _(g: bass.BassEngine):
                g.memzero(zbuf[:])
                g.iota(iota_i[:], pattern=[[1, 32]], base=0, channel_multiplier=0)
                g.tensor_copy(iota_h[:], iota_i[:]).then_inc(sp, 1)
                for i in range(nt):
                    b = i % B
                    k = ks[i]
                    if rs[i] < k:
                        g.wait_ge(sa, exp_done[i])
                        tree_l1(
                            g, et[b], tw[i], k, s16, ALU.add, rs[i], k, sem=sp
                        )
                        tree_rest(
                            g, k, [s16, s8, s4, s2], ALU.add, rs[i], k,
                            S[:, krow0[i] : krow0[i] + k], sem=sp,
                        )

            @block.tensor
            def _(pe: bass.BassEngine):
                n_mm_done = 0
                total_mm = sum(w // 128 for w in tw)
                for i in range(nt):
                    b = i % B
                    bx = tile_buf[i]
                    xo = tile_xoff[i]
                    k = ks[i]
                    eb = tile_eb[i]
                    eo = tile_eoff[i]
                    ko = tile_koff[i]
                    pe.wait_ge(dxs[bx], dx_full[i])
                    pe.wait_ge(sv, H_done[i])
                    xv = (
                        xt[bx][:, xo : xo + tw[i]]
                        .bitcast(bf16)
                        .rearrange("p (w two) -> p w two", two=2)[:, :, 1]
                    )
                    for g_ in range(tw[i] // 128):
                        mm = pe.matmul(
                            psum[:],
                            lhsT=xv[:, g_ * 128 : (g_ + 1) * 128],
                            rhs=Ht[tile_hb[i]][:, eo + g_ * 128 : eo + (g_ + 1) * 128],
                            start=(n_mm_done == 0),
                            stop=(n_mm_done == total_mm - 1),
                        )
                        n_mm_done += 1
                    mm.then_inc(spe, 1)

            @block.vector
            def _(v: bass.BassEngine):
                def emit_H(j):
                    # one-hot H = (iota == td); pair-strided for DVE 2x
                    kj = ks[j]
                    v.wait_ge(sa, tdc_done[tile_tch[j]])
                    if j >= 3 and j <= tail0:
                        v.wait_ge(spe, j - 2)               # Ht[hb] free
                    eoj = tile_eoff[j]
                    v.tensor_tensor(
                        Ht[tile_hb[j]][:, eoj : eoj + tw[j]].rearrange(
                            "p (k s two) -> p k s two", s=16, two=2
                        ),
                        iota_h[:]
                        .rearrange("p (s two) -> p s two", two=2)
                        .unsqueeze(1)
                        .broadcast_to([P, kj, 16, 2]),
                        td[:, 2 * krow0[j] : 2 * (krow0[j] + kj)]
                        .rearrange("p (k two) -> p k two", two=2)
                        .unsqueeze(2)
                        .broadcast_to([P, kj, 16, 2]),
                        op=ALU.is_equal,
                    ).then_inc(sv, 1)

                v.memzero(eq3[0][:])
                v.memzero(eq3[1][:])
                v.memzero(Ht[0][:, 512:1024])
                v.memzero(Ht[1][:, 1536:2048])
                v.wait_ge(sp, 1)  # iota ready
                emit_H(0)
                for i in range(nt):
                    b = tile_eb[i]
                    eo = tile_eoff[i]
                    ko = tile_koff[i]
                    k = ks[i]
                    if i + 1 < nt:
                        emit_H(i + 1)
                    # sum tree rows [0, rs)
                    if i in split_x:
                        kh = k // 2
                        v.wait_ge(sa, exp_done[i] - 1)
                        tree_l1(v, et[b], tw[i], k, s16, ALU.add, 0,
                                min(rs[i], kh), eo=eo)
                        tree_l1(v, et[b], tw[i], k, m16, ALU.max, 0, kh,
                                eo=eo)
                        v.wait_ge(sa, exp_done[i])
                        tree_l1(v, et[b], tw[i], k, s16, ALU.add,
                                min(rs[i], kh), rs[i], eo=eo)
                        tree_l1(v, et[b], tw[i], k, m16, ALU.max, kh, k,
                                eo=eo)
                    else:
                        v.wait_ge(sa, exp_done[i])
                        tree_l1(v, et[b], tw[i], k, s16, ALU.add, 0, rs[i],
                                eo=eo)
                        tree_l1(v, et[b], tw[i], k, m16, ALU.max, 0, k,
                                eo=eo)
                    tree_rest(
                        v, k, [s16, s8, s4, s2], ALU.add, 0, rs[i],
                        S[:, krow0[i] : krow0[i] + k], sem=sv,
                    )
                    tree_rest(v, k, [m16, m8, m4, m2], ALU.max, 0, k, M)
                    # flag indicator: eq3 = (e3 == rowmax); PE reduces it
                    e3d = et[b][:, eo : eo + tw[i]].rearrange(
                        "p (k c) -> p k c", c=32
                    )
                    v.tensor_tensor(
                        eq3[b][:, ko : ko + k], e3d[:, :, 3], M[:, 0:k],
                        op=ALU.is_equal,
                    )
                    v.scalar_tensor_tensor(
                        fjunk[:, 0:k],
                        eq3[b][:, ko : ko + k],
                        1.0,
                        Ht[tile_hb[i]][:, eo : eo + tw[i]].rearrange(
                            "p (k c) -> p k c", c=32
                        )[:, :, 2],
                        op0=ALU.mult,
                        op1=ALU.mult,
                        accum_out=out_sb[
                            :, 256 + n_lnch + i : 257 + n_lnch + i
                        ],
                    ).then_inc(sv, 1)
                    if dbg_tile == i:
                        v.tensor_copy(dbg_sb[:, 0:KMAX], eq3[b][:, 0:KMAX])
                        v.tensor_copy(dbg_sb[:, KMAX : KMAX + k], M[:, 0:k])
                        v.tensor_copy(dbg_sb[:, 2 * KMAX : 2 * KMAX + k],
                                      e3d[:, :, 3])
                v.memzero(out_sb[:, 128:256])
                v.sem_inc(sv, 1)

    return nc


def _get_nc():
    key = "nc"
    if key not in _CACHE:
        _CACHE[key] = _build_nc()
    return _CACHE[key]


def _finish(out_list, epoch) -> np.float32:
    """Host-side final scalar arithmetic from per-core partials."""
    n_lnch = len(LN_CHUNKS)
    lnsum = 0.0
    flagsum = 0.0
    picksum = 0.0
    for o in out_list:
        o64 = o.astype(np.float64)
        picksum += np.trace(o64[:, 0:128])
        lnsum += o64[:, 256 : 256 + n_lnch].sum()
        flagsum += o64[:, 256 + n_lnch : 256 + n_lnch + NT].sum()
    init_loss = (lnsum - picksum) / N
    corr = float(epoch) ** (-0.65) * 64.0 + 0.01
    loss = init_loss + (corr if flagsum > 0.5 else 0.0)
    bad = (loss < 0) or (loss / init_loss < 0.2)
    return np.float32(init_loss if bad else loss)


def kernel(output: np.ndarray, target: np.ndarray, epoch) -> np.ndarray:
    from concourse.bass_utils import run_bass_kernel_spmd

    nc = _get_nc()

    output = np.ascontiguousarray(output, dtype=np.float32)
    t8 = np.ascontiguousarray(target, dtype=np.int64).astype(np.uint8)

    in_maps = []
    for cid in range(NCORES):
        xs = output[cid * NSH : (cid + 1) * NSH]
        ts = t8[cid * NSH : (cid + 1) * NSH]
        in_maps.append(
            {
                "x": xs.reshape(P, KTOT * C),
                "t8": ts.reshape(P, KTOT),
            }
        )

    res = run_bass_kernel_spmd(nc, in_maps, list(range(NCORES)))
    return _finish([res.results[i]["out"] for i in range(NCORES)], epoch)

